# revision 9
# baseline (speedup 1.0000x reference)
# DigitCaps (capsule routing) Trainium2 kernel, SPMD over 8 NeuronCores.
#
# Reference computation:
#   s_hat[b,j,n,o] = sum_i s[b,n,i] * W[j,n,o,i]
#   b0 = 0
#   for k in range(R):  c = softmax_j(b_k); v_k = squash(sum_n c * s_hat)
#                       b_{k+1} = b_k + sum_o v_k * s_hat
#
# Strategy (N-sharded, n_c = 256 per core; softmax over j stays core-local):
# s_hat (536 MB) is never materialized. Each routing sweep is expressed as
# TensorEngine contractions against W directly:
#   * sweep 0 (c uniform): s_v0 = (1/J) sum_{n,i} s*W -- folded matmul,
#     W1 streamed as the moving operand at full rate (F=512).
#   * logits b_k[b,j,n] = sum_o vc[b,j,o]*s_hat[...], vc = sum_{m<=k} v_m
#     (cumulative), via block-diagonal stationary Vbd[(jg,o),(jg,b)] against
#     W2[(jg,o),(g,n,i)] giving U[b,j,n,i] in PSUM, then U*s (DVE) and a
#     reduce over i.
#   * s_v_k = sum_{n,i} (c_k ⊙ s) * W1 per output capsule j (c folded into
#     the small stationary operand X_j).
# W1 bf16 [n | no,j,i,o] is SBUF-resident (16.75 MB); W2 bf16 [(jg,o)|g,n,i]
# is streamed from DRAM per sweep. Partial s_v is AllReduced (262 KB) per
# sweep; every core then computes the identical squash.
#
# Host side pre-shards / pre-transposes / bf16-casts inputs into the exact
# SBUF layouts.

from contextlib import ExitStack

import numpy as np
import ml_dtypes

import concourse.bass as bass
import concourse.mybir as mybir
import concourse.tile as tile
from concourse import bacc
from concourse.bass_utils import run_bass_kernel_spmd

NCORES = 8
B, N, DI, J, DO = 32, 2048, 16, 64, 32
NC_ = N // NCORES          # 256 n per core
NO = NC_ // 128            # 2 n-chunks of 128
NG = J // 4                # 16 j-groups of 4

_cache = {}


def _body(ctx, nc, tc, R, s1_d, s2_d, w1_d, w2_d, out_d, no_collective=False):
    f32 = mybir.dt.float32
    bf16 = mybir.dt.bfloat16
    AT = mybir.ActivationFunctionType
    OP = mybir.AluOpType
    X = mybir.AxisListType.X

    resident = ctx.enter_context(tc.tile_pool(name="resident", bufs=1))
    small = ctx.enter_context(tc.tile_pool(name="small", bufs=1))
    dram = ctx.enter_context(tc.tile_pool(name="dram", bufs=1, space="DRAM"))

    # ---- resident SBUF state ----
    w1 = resident.tile([128, NO, J, DI, DO], bf16)   # 128 KB/partition
    s1 = resident.tile([128, NO, DI, B], bf16)       # s, n on partitions
    s2 = resident.tile([128, NO, 128, DI], bf16)     # s, replicated over jg
    vbd = resident.tile([128, NG, 128], bf16)        # block-diag vc
    sv = resident.tile([B, J, DO], f32)              # s_v partial / v
    vc = resident.tile([B, J, DO], bf16)             # cumulative v
    blgT = resident.tile([128, NO, J, B], bf16)      # logits -> exp -> c

    nc.sync.dma_start(s1[:], s1_d)
    nc.sync.dma_start(s2[:], s2_d)
    for q in range(8):
        nc.sync.dma_start(w1[:, :, 8 * q:8 * (q + 1)], w1_d[:, :, 8 * q:8 * (q + 1)])
    nc.vector.memzero(vbd[:])

    ar_in = dram.tile([B, J * DO], f32)
    ar_out = dram.tile([B, J * DO], f32)

    def allreduce_and_squash():
        """sv: partial s_v -> allreduce -> squash in place (sv becomes v)."""
        nc.sync.dma_start(ar_in[:], sv[:].rearrange("b j o -> b (j o)"))
        if no_collective:
            nc.sync.dma_start(ar_out[:], ar_in[:])
        else:
            nc.gpsimd.collective_compute(
                "AllReduce", OP.add,
                replica_groups=[list(range(NCORES))],
                ins=[ar_in[:].opt()],
                outs=[ar_out[:].opt()],
            )
        nc.sync.dma_start(sv[:].rearrange("b j o -> b (j o)"), ar_out[:])
        # squash: v = sv * sq / ((1+sq) * sqrt(sq + 1e-7)),  sq = sum_o sv^2
        t2 = small.tile([B, J, DO], f32, tag="sq_tmp")
        nc.vector.tensor_tensor(t2[:], sv[:], sv[:], OP.mult)
        sq = small.tile([B, J], f32, tag="sq")
        nc.vector.tensor_reduce(sq[:], t2[:], X, OP.add)
        sqe = small.tile([B, J], f32, tag="sqe")
        nc.vector.tensor_scalar_add(sqe[:], sq[:], 1e-7)
        rt = small.tile([B, J], f32, tag="rt")
        nc.scalar.activation(rt[:], sqe[:], AT.Sqrt)
        u = small.tile([B, J], f32, tag="u")
        nc.vector.scalar_tensor_tensor(u[:], sq[:], 1.0, rt[:], OP.add, OP.mult)
        rc = small.tile([B, J], f32, tag="rc")
        nc.vector.reciprocal(rc[:], u[:])
        fac = small.tile([B, J], f32, tag="fac")
        nc.vector.tensor_tensor(fac[:], sq[:], rc[:], OP.mult)
        nc.vector.tensor_tensor(
            sv[:], sv[:], fac[:].unsqueeze(2).to_broadcast((B, J, DO)), OP.mult)

    # ================= sweep 0: uniform c -> folded matmul =================
    with tc.tile_pool(name="ps0", bufs=1, space="PSUM") as ps0:
        psum0 = ps0.tile([B, 4, 512], f32)
        nsteps = NO * DI
        for st in range(nsteps):
            no, i = divmod(st, DI)
            lhsT = s1[:, no, i, :]                       # [128, B]
            for f in range(4):
                rhs = w1[:, no, 16 * f:16 * (f + 1), i, :]   # [128,16,DO]=512
                nc.tensor.matmul(psum0[:, f], lhsT, rhs,
                                 start=(st == 0), stop=(st == nsteps - 1))
        nc.vector.tensor_scalar_mul(
            sv[:].rearrange("b j o -> b (j o)"),
            psum0[:].rearrange("b f x -> b (f x)"), 1.0 / J)

    allreduce_and_squash()                               # sv now holds v_0
    if R == 1:
        nc.sync.dma_start(out_d, sv[:])
        return
    nc.vector.tensor_copy(out=vc[:], in_=sv[:])

    for k in range(1, R):
        # ---- build Vbd (block-diagonal vc^T) ----
        vct = small.tile([DO, J, B], bf16, tag="vct")
        nc.vector.transpose(vct[:].rearrange("o j b -> o (j b)"),
                            vc[:].rearrange("b j o -> b (j o)"))
        with nc.allow_non_contiguous_dma(reason="tiny Vbd diagonal blocks"):
            for g in range(NG):
                for jg in range(4):
                    nc.sync.dma_start(
                        vbd[32 * jg:32 * (jg + 1), g, 32 * jg:32 * (jg + 1)],
                        vct[:, 4 * g + jg, :])

        # ---- (A): logits = reduce_i(s2 * (Vbd^T @ W2)) ----
        with tc.tile_pool(name="w2p", bufs=3) as w2p, \
             tc.tile_pool(name="psU", bufs=2, space="PSUM") as psU, \
             tc.tile_pool(name="mats", bufs=2) as mats:
            for g in range(NG):
                for no in range(NO):
                    w2t = w2p.tile([128, 128, DI], bf16, tag="w2t")
                    nc.sync.dma_start(w2t[:], w2_d[:, g, 128 * no:128 * (no + 1)])
                    pU = psU.tile([128, 4, 512], f32, tag="pU")
                    for f in range(4):
                        rhs = w2t[:, 32 * f:32 * (f + 1), :]     # [128,32,16]=512
                        nc.tensor.matmul(pU[:, f], vbd[:, g, :], rhs,
                                         start=True, stop=True)
                    m = mats.tile([128, 128, DI], bf16, tag="m")
                    nc.vector.tensor_tensor(
                        m[:], pU[:].rearrange("p f x -> p (f x)")
                                   .rearrange("p (n i) -> p n i", i=DI),
                        s2[:, no], OP.mult)
                    blg = mats.tile([128, 128], f32, tag="blg")
                    nc.vector.tensor_reduce(blg[:], m[:], X, OP.add)
                    blgb = mats.tile([128, 128], bf16, tag="blgb")
                    nc.scalar.copy(blgb[:], blg[:])
                    # transpose [(jg,b), n] -> [n, (jg,b)]; j = 4g+jg
                    nc.sync.dma_start_transpose(
                        blgT[:, no, 4 * g:4 * (g + 1), :]
                            .rearrange("p a b -> p (a b)"),
                        blgb[:])

        # ---- softmax over j (n on partitions, fully local) ----
        nc.scalar.activation(blgT[:], blgT[:], AT.Exp)
        den = small.tile([128, NO, B], f32, tag="den")
        nc.vector.tensor_reduce(den[:], blgT[:].transpose([0, 1, 3, 2]), X, OP.add)
        rden = small.tile([128, NO, B], bf16, tag="rden")
        with nc.allow_low_precision(reason="softmax denom in bf16, tol 2e-2"):
            nc.vector.reciprocal(rden[:], den[:])
        nc.vector.tensor_tensor(
            blgT[:], blgT[:],
            rden[:].unsqueeze(2).to_broadcast((128, NO, J, B)), OP.mult)
        # blgT now holds c[n | no, j, b]

        # ---- (b): s_v[b,j,o] = sum_{n,i} (c_j ⊙ s) W1_j ----
        with tc.tile_pool(name="xp", bufs=4) as xp, \
             tc.tile_pool(name="psB", bufs=8, space="PSUM") as psB:
            for j in range(J):
                xj = xp.tile([128, NO, DI, B], bf16, tag="xj")
                nc.vector.tensor_tensor(
                    xj[:], s1[:],
                    blgT[:, :, j, :].unsqueeze(2)
                        .to_broadcast((128, NO, DI, B)), OP.mult)
                pB = psB.tile([B, DO], f32, tag="pB")
                nsteps = NO * DI
                for st in range(nsteps):
                    no, i = divmod(st, DI)
                    nc.tensor.matmul(pB[:], xj[:, no, i, :], w1[:, no, j, i, :],
                                     start=(st == 0), stop=(st == nsteps - 1))
                nc.scalar.copy(sv[:, j, :], pB[:])

        allreduce_and_squash()                           # sv now holds v_k
        if k == R - 1:
            nc.sync.dma_start(out_d, sv[:])
        else:
            nc.vector.tensor_tensor(vc[:], vc[:], sv[:], OP.add)


def _build(R, no_collective=False):
    nc = bacc.Bacc("TRN2", target_bir_lowering=False, debug=False,
                   num_devices=1 if no_collective else NCORES)
    f32, bf16 = mybir.dt.float32, mybir.dt.bfloat16
    s1_d = nc.dram_tensor("s1", [128, NO, DI, B], bf16, kind="ExternalInput").ap()
    s2_d = nc.dram_tensor("s2", [128, NO, 128, DI], bf16, kind="ExternalInput").ap()
    w1_d = nc.dram_tensor("w1", [128, NO, J, DI, DO], bf16,
                          kind="ExternalInput").ap()
    w2_d = nc.dram_tensor("w2", [128, NG, NC_, DI], bf16, kind="ExternalInput").ap()
    out_d = nc.dram_tensor("v_out", [B, J, DO], f32, kind="ExternalOutput").ap()
    with tile.TileContext(nc) as tc:
        with ExitStack() as ctx:
            _body(ctx, nc, tc, R, s1_d, s2_d, w1_d, w2_d, out_d,
                  no_collective=no_collective)
    nc.compile()
    return nc


def _shard_inputs(s, W):
    """Host-side: slice per core and lay out into the SBUF-ready formats."""
    bf = ml_dtypes.bfloat16
    in_maps = []
    for k in range(NCORES):
        ns = k * NC_
        s_sl = s[:, ns:ns + NC_, :]                    # [B, 256, DI]
        W_sl = W[:, ns:ns + NC_, :, :]                 # [J, 256, DO, DI]
        # s1[p, no, i, b] = s_sl[b, no*128+p, i]
        s1 = np.ascontiguousarray(
            s_sl.transpose(1, 2, 0).reshape(NO, 128, DI, B).transpose(1, 0, 2, 3)
        ).astype(bf)
        # s2[32*jg+b, no, n, i] = s_sl[b, no*128+n, i]
        s2 = np.ascontiguousarray(
            np.tile(s_sl.reshape(B, NO, 128, DI), (4, 1, 1, 1))).astype(bf)
        # w1[p, no, j, i, o] = W_sl[j, no*128+p, o, i]
        w1 = np.ascontiguousarray(
            W_sl.transpose(1, 0, 3, 2).reshape(NO, 128, J, DI, DO)
            .transpose(1, 0, 2, 3, 4)).astype(bf)
        # w2[32*jg+o, g, n, i] = W_sl[4g+jg, n, o, i]
        w2 = np.ascontiguousarray(
            W_sl.reshape(NG, 4, NC_, DO, DI).transpose(1, 3, 0, 2, 4)
            .reshape(128, NG, NC_, DI)).astype(bf)
        in_maps.append({"s1": s1, "s2": s2, "w1": w1, "w2": w2})
    return in_maps


def kernel(s, W, num_routing):
    R = int(num_routing)
    assert R >= 1
    s = np.asarray(s, dtype=np.float32)
    W = np.asarray(W, dtype=np.float32)
    if R not in _cache:
        _cache[R] = _build(R)
    nc = _cache[R]
    in_maps = _shard_inputs(s, W)
    res = run_bass_kernel_spmd(nc, in_maps, core_ids=list(range(NCORES)))
    return res.results[0]["v_out"].astype(np.float32)
